# revision 8
# baseline (speedup 1.0000x reference)
"""AttnBlock (GroupNorm + single-head self-attention + residual) for TRN2.

Contract: kernel(**inputs) takes the FULL inputs from setup_inputs() and
returns the FULL [2, 512, 64, 64] output.

Distribution: 8 cores = 2 batches x 4 query-chunks of 1024 tokens each.
Each core redundantly computes GroupNorm + K/V for its batch (cheap vs
attention) and attends its own 1024 query columns.

Layout strategy (no transposes anywhere):
  - everything lives as [channel-partition, token-free] tiles
  - scores are computed transposed: sT[j,i] = sum_o k[o,j] q[o,i]
  - softmax without max-subtraction (scores ~ N(0,1); exp is fp32-safe),
    so exp is elementwise and layout-free
  - row sums l[i] come from a ones column appended to vT in the PV matmul
  - v-bias and the 1/l normalization are deferred past the output
    projection:  x + wp @ (acc/l + bv) + bp = x + (wp@acc)*rl + (wp@bv + bp)
All heavy matmuls run as float32r (fp32 data, full PE rate).
"""

import numpy as np
from contextlib import ExitStack

import concourse.bass as bass
import concourse.bacc as bacc
import concourse.tile as tile
from concourse import mybir
from concourse.bass_utils import run_bass_kernel_spmd

F32 = mybir.dt.float32
F32R = mybir.dt.float32r
AL = mybir.AluOpType
AF = mybir.ActivationFunctionType

B = 2
C = 512
N = 4096          # tokens per batch (64*64)
NQ = 1024         # query tokens per core
P = 128
NCC = C // P      # 4 channel chunks
G = 32            # groups
EPS = 1e-6
NIH = NQ // 512   # 2 i-halves of 512
NJB = N // 512    # 8 j-blocks of 512
SCALE = float(C) ** -0.5


def _r(ap):
    """Bitcast an fp32 SBUF AP to float32r for full-rate matmuls."""
    return ap.bitcast(F32R)


def build_nc():
    nc = bacc.Bacc(None, target_bir_lowering=False)

    xf = nc.dram_tensor("xf", [C, N], F32R, kind="ExternalInput")
    xq = nc.dram_tensor("xq", [C, NQ], F32R, kind="ExternalInput")
    wqt = nc.dram_tensor("wqt", [C, C], F32R, kind="ExternalInput")
    wkt = nc.dram_tensor("wkt", [C, C], F32R, kind="ExternalInput")
    wvt = nc.dram_tensor("wvt", [C, C], F32R, kind="ExternalInput")
    wpt = nc.dram_tensor("wpt", [C, C], F32R, kind="ExternalInput")
    cb = nc.dram_tensor("cb", [C, 3], F32, kind="ExternalInput")    # bq*s, bk, wp@bv+bp
    gaff = nc.dram_tensor("gaff", [C, 2], F32, kind="ExternalInput")  # gn_scale, gn_bias
    gm = nc.dram_tensor("gm", [C, G], F32, kind="ExternalInput")    # indicator/16
    gmt = nc.dram_tensor("gmt", [G, C], F32, kind="ExternalInput")  # indicator
    out = nc.dram_tensor("out", [C, NQ], F32, kind="ExternalOutput")

    with tile.TileContext(nc) as tc, ExitStack() as ctx:
        const = ctx.enter_context(tc.tile_pool(name="const", bufs=1))
        wpool = ctx.enter_context(tc.tile_pool(name="wpool", bufs=1))
        hx = ctx.enter_context(tc.tile_pool(name="hx", bufs=1))
        qx = ctx.enter_context(tc.tile_pool(name="qx", bufs=1))
        xqp = ctx.enter_context(tc.tile_pool(name="xqp", bufs=1))
        kbp = ctx.enter_context(tc.tile_pool(name="kbp", bufs=2))
        vbp = ctx.enter_context(tc.tile_pool(name="vbp", bufs=2))
        ptp = ctx.enter_context(tc.tile_pool(name="ptp", bufs=3))
        osp = ctx.enter_context(tc.tile_pool(name="osp", bufs=1))
        tmp = ctx.enter_context(tc.tile_pool(name="tmp", bufs=2))
        mmp = ctx.enter_context(tc.tile_pool(name="mmp", bufs=3, space="PSUM"))
        accp = ctx.enter_context(tc.tile_pool(name="accp", bufs=1, space="PSUM"))
        lpp = ctx.enter_context(tc.tile_pool(name="lpp", bufs=1, space="PSUM"))

        # ---- constant / weight loads ----
        w_sb = {}
        for wname, wdram in (("q", wqt), ("k", wkt), ("v", wvt), ("p", wpt)):
            for cc in range(NCC):
                t = wpool.tile([P, C], F32R, tag=f"w{wname}{cc}")
                nc.sync.dma_start(out=t[:], in_=wdram[cc * P:(cc + 1) * P, :])
                w_sb[wname, cc] = t
        cb_sb = []
        gaff_sb = []
        gm_sb = []
        for cc in range(NCC):
            t = const.tile([P, 3], F32, tag=f"cb{cc}")
            nc.sync.dma_start(out=t[:], in_=cb[cc * P:(cc + 1) * P, :])
            cb_sb.append(t)
            t = const.tile([P, 2], F32, tag=f"ga{cc}")
            nc.sync.dma_start(out=t[:], in_=gaff[cc * P:(cc + 1) * P, :])
            gaff_sb.append(t)
            t = const.tile([P, G], F32, tag=f"gm{cc}")
            nc.sync.dma_start(out=t[:], in_=gm[cc * P:(cc + 1) * P, :])
            gm_sb.append(t)
        gmt_sb = const.tile([G, C], F32, tag="gmt")
        nc.sync.dma_start(out=gmt_sb[:], in_=gmt[:, :])
        eps_sb = const.tile([G, 1], F32, tag="eps")
        nc.vector.memset(eps_sb[:], EPS)
        ones_sb = const.tile([1, P], F32, tag="ones")
        nc.vector.memset(ones_sb[:], 1.0)
        onescol_sb = const.tile([P, 1], F32, tag="onescol")
        nc.vector.memset(onescol_sb[:], 1.0)

        # ---- load x (batch) and xq (query cols); GroupNorm stats ----
        h_sb = []     # holds x, then h in place
        for cc in range(NCC):
            t = hx.tile([P, N], F32R, tag=f"h{cc}")
            nc.sync.dma_start(out=t[:], in_=xf[cc * P:(cc + 1) * P, :])
            h_sb.append(t)
        xq_sb = []    # holds xq, then h_q in place
        for cc in range(NCC):
            t = xqp.tile([P, NQ], F32R, tag=f"xq{cc}")
            nc.sync.dma_start(out=t[:], in_=xq[cc * P:(cc + 1) * P, :])
            xq_sb.append(t)

        # per-channel mean/var via bn_stats over the 4096 free dim
        agg_ps = mmp.tile([G, 2], F32, tag="mm")
        for cc in range(NCC):
            xv = h_sb[cc][:].bitcast(F32).rearrange("p (s f) -> p s f", f=512)
            stats = tmp.tile([P, 8, 6], F32, tag="bst")
            for s in range(8):
                nc.vector.bn_stats(out=stats[:, s, :], in_=xv[:, s, :])
            mv = tmp.tile([P, 2], F32, tag="mv")
            nc.vector.bn_aggr(out=mv[:], in_=stats[:])
            # mu = [mean_c, var_c + mean_c^2]
            mu = tmp.tile([P, 2], F32, tag=f"mu{cc}")
            nc.vector.tensor_copy(mu[:, 0:1], mv[:, 0:1])
            nc.vector.scalar_tensor_tensor(
                out=mu[:, 1:2], in0=mv[:, 0:1], scalar=mv[:, 0:1],
                in1=mv[:, 1:2], op0=AL.mult, op1=AL.add)
            # group aggregation: [32,2] += gm[cc].T @ mu   (values 1/16)
            nc.tensor.matmul(out=agg_ps[:], lhsT=gm_sb[cc][:], rhs=mu[:],
                             start=(cc == 0), stop=(cc == NCC - 1))
        # grs = [mean_g, rstd_g]
        eg = tmp.tile([G, 2], F32, tag="eg")
        nc.vector.tensor_copy(eg[:], agg_ps[:])
        msq = tmp.tile([G, 1], F32, tag="msq")
        nc.vector.tensor_mul(msq[:], eg[:, 0:1], eg[:, 0:1])
        grs = tmp.tile([G, 2], F32, tag="grs")
        nc.vector.tensor_copy(grs[:, 0:1], eg[:, 0:1])
        var = tmp.tile([G, 1], F32, tag="var")
        nc.vector.tensor_sub(var[:], eg[:, 1:2], msq[:])
        std = tmp.tile([G, 1], F32, tag="std")
        nc.scalar.activation(out=std[:], in_=var[:], func=AF.Sqrt, bias=eps_sb[:])
        nc.vector.reciprocal(grs[:, 1:2], std[:])

        # broadcast back to channels; per-channel affine a=rstd*gns, b=gnb-mean*a
        ab_sb = []
        for cc in range(NCC):
            bc_ps = mmp.tile([P, 2], F32, tag="mm")
            nc.tensor.matmul(out=bc_ps[:],
                             lhsT=gmt_sb[:, cc * P:(cc + 1) * P], rhs=grs[:],
                             start=True, stop=True)
            ab = const.tile([P, 2], F32, tag=f"ab{cc}")
            nc.vector.tensor_mul(ab[:, 0:1], bc_ps[:, 1:2], gaff_sb[cc][:, 0:1])
            t2 = tmp.tile([P, 1], F32, tag="t2")
            nc.vector.tensor_mul(t2[:], bc_ps[:, 0:1], ab[:, 0:1])
            nc.vector.tensor_sub(ab[:, 1:2], gaff_sb[cc][:, 1:2], t2[:])
            ab_sb.append(ab)

        # h = a*x + b  (in place), and h_q on the query columns
        for cc in range(NCC):
            nc.vector.tensor_scalar(
                out=h_sb[cc][:], in0=h_sb[cc][:].bitcast(F32),
                scalar1=ab_sb[cc][:, 0:1], scalar2=ab_sb[cc][:, 1:2],
                op0=AL.mult, op1=AL.add)
            nc.vector.tensor_scalar(
                out=xq_sb[cc][:], in0=xq_sb[cc][:].bitcast(F32),
                scalar1=ab_sb[cc][:, 0:1], scalar2=ab_sb[cc][:, 1:2],
                op0=AL.mult, op1=AL.add)

        # ---- q projection (q = wq*s @ h_q + bq*s), kept resident ----
        q_sb = []
        for oc in range(NCC):
            t = qx.tile([P, NQ], F32R, tag=f"q{oc}")
            q_sb.append(t)
        for ih in range(NIH):
            isl = slice(ih * 512, (ih + 1) * 512)
            for oc in range(NCC):
                ps = mmp.tile([P, 512], F32, tag="mm")
                for cc in range(NCC):
                    nc.tensor.matmul(
                        out=ps[:],
                        lhsT=w_sb["q", cc][:, oc * P:(oc + 1) * P],
                        rhs=xq_sb[cc][:, isl],
                        start=(cc == 0), stop=(cc == NCC - 1))
                nc.vector.tensor_scalar(
                    out=q_sb[oc][:, isl], in0=ps[:],
                    scalar1=cb_sb[oc][:, 0:1], scalar2=None,
                    op0=AL.add)  # writes f32r (tile dtype)

        # reload raw xq for the residual (slot reuse; waits for q-proj reads)
        xqr_sb = []
        for cc in range(NCC):
            t = xqp.tile([P, NQ], F32R, tag=f"xq{cc}")
            nc.sync.dma_start(out=t[:], in_=xq[cc * P:(cc + 1) * P, :])
            xqr_sb.append(t)

        # ---- attention ----
        for ih in range(NIH):
            isl = slice(ih * 512, (ih + 1) * 512)
            acc_ps = []
            for cv in range(NCC):
                acc_t = accp.tile([P, 512], F32, tag=f"acc{cv}")
                acc_ps.append(acc_t)
            l_ps = lpp.tile([1, 512], F32, tag="l")
            for jb in range(NJB):
                jsl = slice(jb * 512, (jb + 1) * 512)
                # k block: [128o, 512j] per output chunk
                kb = []
                for oc in range(NCC):
                    ps = mmp.tile([P, 512], F32, tag="mm")
                    for cc in range(NCC):
                        nc.tensor.matmul(
                            out=ps[:],
                            lhsT=w_sb["k", cc][:, oc * P:(oc + 1) * P],
                            rhs=h_sb[cc][:, jsl],
                            start=(cc == 0), stop=(cc == NCC - 1))
                    t = kbp.tile([P, 512], F32R, tag=f"kb{oc}")
                    nc.vector.tensor_scalar(
                        out=t[:], in0=ps[:], scalar1=cb_sb[oc][:, 1:2],
                        scalar2=None, op0=AL.add)
                    kb.append(t)
                # vT block: [128j, 512c' + ones] per j-tile
                vb = []
                for jt in range(4):
                    g = jb * 4 + jt
                    ps = mmp.tile([P, 512], F32, tag="mm")
                    for cc in range(NCC):
                        nc.tensor.matmul(
                            out=ps[:],
                            lhsT=h_sb[cc][:, g * P:(g + 1) * P],
                            rhs=w_sb["v", cc][:, :],
                            start=(cc == 0), stop=(cc == NCC - 1))
                    t = vbp.tile([P, C + 1], F32R, tag=f"vb{jt}")
                    nc.vector.tensor_copy(t[:, 0:C], ps[:])
                    nc.vector.tensor_copy(t[:, C:C + 1], onescol_sb[:])
                    vb.append(t)
                # scores^T, exp, PV accumulate
                for jt in range(4):
                    first = (jb == 0 and jt == 0)
                    last = (jb == NJB - 1 and jt == 3)
                    ps = mmp.tile([P, 512], F32, tag="mm")
                    for oc in range(NCC):
                        nc.tensor.matmul(
                            out=ps[:],
                            lhsT=kb[oc][:, jt * P:(jt + 1) * P],
                            rhs=q_sb[oc][:, isl],
                            start=(oc == 0), stop=(oc == NCC - 1))
                    pt = ptp.tile([P, 512], F32R, tag="pt")
                    nc.scalar.activation(out=pt[:], in_=ps[:], func=AF.Exp)
                    for cv in range(NCC):
                        nc.tensor.matmul(
                            out=acc_ps[cv][:],
                            lhsT=vb[jt][:, cv * P:(cv + 1) * P],
                            rhs=pt[:],
                            start=first, stop=last)
                    nc.tensor.matmul(
                        out=l_ps[:], lhsT=vb[jt][:, C:C + 1], rhs=pt[:],
                        start=first, stop=last)

            # ---- epilogue for this i-half ----
            l_sb = tmp.tile([1, 512], F32, tag="lsb")
            nc.vector.tensor_copy(l_sb[:], l_ps[:])
            rl = tmp.tile([1, 512], F32, tag="rl")
            nc.vector.reciprocal(rl[:], l_sb[:])
            rlb_ps = mmp.tile([P, 512], F32, tag="mm")
            nc.tensor.matmul(out=rlb_ps[:], lhsT=ones_sb[:], rhs=rl[:],
                             start=True, stop=True)
            rlb = tmp.tile([P, 512], F32, tag="rlb")
            nc.vector.tensor_copy(rlb[:], rlb_ps[:])
            os_sb = []
            for cv in range(NCC):
                t = osp.tile([P, 512], F32R, tag=f"os{cv}")
                nc.vector.tensor_copy(t[:], acc_ps[cv][:])
                os_sb.append(t)
            for oc in range(NCC):
                ps = mmp.tile([P, 512], F32, tag="mm")
                for cv in range(NCC):
                    nc.tensor.matmul(
                        out=ps[:],
                        lhsT=w_sb["p", cv][:, oc * P:(oc + 1) * P],
                        rhs=os_sb[cv][:],
                        start=(cv == 0), stop=(cv == NCC - 1))
                fin = tmp.tile([P, 512], F32, tag="fin")
                nc.vector.tensor_mul(fin[:], ps[:], rlb[:])
                nc.vector.scalar_tensor_tensor(
                    out=fin[:], in0=fin[:], scalar=cb_sb[oc][:, 2:3],
                    in1=xqr_sb[oc][:, isl].bitcast(F32), op0=AL.add, op1=AL.add)
                nc.sync.dma_start(out=out[oc * P:(oc + 1) * P, isl], in_=fin[:])

    nc.compile()
    return nc


_NC = None


def _get_nc():
    global _NC
    if _NC is None:
        _NC = build_nc()
    return _NC


def make_in_maps(x, gn_scale, gn_bias, wq, bq, wk, bk, wv, bv, wp, bp):
    f = np.float32
    x = np.asarray(x, f)
    wq = np.asarray(wq, f); wk = np.asarray(wk, f)
    wv = np.asarray(wv, f); wp = np.asarray(wp, f)
    bq = np.asarray(bq, f); bk = np.asarray(bk, f)
    bv = np.asarray(bv, f); bp = np.asarray(bp, f)
    gn_scale = np.asarray(gn_scale, f); gn_bias = np.asarray(gn_bias, f)

    wqt = np.ascontiguousarray(wq.T) * np.float32(SCALE)
    wkt = np.ascontiguousarray(wk.T)
    wvt = np.ascontiguousarray(wv.T)
    wpt = np.ascontiguousarray(wp.T)
    cp = wp.astype(np.float64) @ bv.astype(np.float64) + bp
    cb = np.stack([bq * np.float32(SCALE), bk, cp.astype(f)], axis=1)
    cb = np.ascontiguousarray(cb, f)
    gaff = np.ascontiguousarray(np.stack([gn_scale, gn_bias], axis=1), f)
    gmat = np.zeros((C, G), f)
    gmat[np.arange(C), np.arange(C) // (C // G)] = 1.0 / (C // G)
    gmatt = np.zeros((G, C), f)
    gmatt[np.arange(C) // (C // G), np.arange(C)] = 1.0

    in_maps = []
    for b in range(B):
        xb = np.ascontiguousarray(x[b].reshape(C, N))
        for qc in range(N // NQ):
            xqc = np.ascontiguousarray(xb[:, qc * NQ:(qc + 1) * NQ])
            in_maps.append(dict(
                xf=xb, xq=xqc, wqt=wqt, wkt=wkt, wvt=wvt, wpt=wpt,
                cb=cb, gaff=gaff, gm=gmat, gmt=gmatt))
    return in_maps


def assemble(results, x):
    outf = np.empty((B, C, N), np.float32)
    i = 0
    for b in range(B):
        for qc in range(N // NQ):
            outf[b, :, qc * NQ:(qc + 1) * NQ] = results[i]["out"]
            i += 1
    return outf.reshape(x.shape)


def kernel(x, gn_scale, gn_bias, wq, bq, wk, bk, wv, bv, wp, bp, **run_kwargs):
    nc = _get_nc()
    in_maps = make_in_maps(x, gn_scale, gn_bias, wq, bq, wk, bk, wv, bv, wp, bp)
    res = run_bass_kernel_spmd(nc, in_maps, core_ids=list(range(8)), **run_kwargs)
    out = assemble(res.results, np.asarray(x))
    if run_kwargs:
        return out, res
    return out
